# revision 1
# baseline (speedup 1.0000x reference)
"""LSTM decoder kernel for Trainium2 (8 NeuronCores, pure data parallel).

Problem: 25-step autoregressive LSTM decode, BATCH=262144, POSE=16, H=32.
  reference: per step  gates = x@W_ih.T + h@W_hh.T + b;  i,f,g,o = split(gates)
             c = sig(f)*c + sig(i)*tanh(g); h = sig(o)*tanh(c); x = h@W_out.T + b_out

Kernel design (per core, B_loc = 32768 rows):
  * Projection folded into the recurrence:  for t>=1
        gates_t = h_t @ W_eff.T + b_eff,   W_eff = W_ih@W_out + W_hh,
        b_eff = b_ih + b_hh + W_ih@b_out
    so the recurrence needs ONE K=32 matmul per step; the pose-space outputs
    x_t = h_t @ W_out.T (+ b_out added on host) are computed on the side.
  * hidden-on-partitions "strip" layout: h/c live as [128 = 4 strips x 32 hidden,
    batch-cols].  Strip x holds a contiguous 2048-row block of the run's batch.
  * Gate matmuls: diagonal PE tiles tile_position=(32x,32x), one round per gate
    type (i,f,g,o), each round = 4 concurrent K=32,M=32,N=512 matmuls -> each
    PSUM bank holds one gate type for all 4 strips, chunk-aligned.  Gate biases
    ride in ACTIVATE's per-partition bias operand.
  * Projection: lhsT = h slice (already transposed in this layout!) -> x tiles
    come out BATCH-major [128 batch, 16] in PSUM; DVE copies into an SBUF
    staging buffer; one big 1600B-contiguous DMA per run writes DRAM.
  * dtypes: matmuls/h/c/gate activations bf16 (l2 rel err ~1.6e-3), PSUM fp32.
"""

import os
import numpy as np
import ml_dtypes

bf16 = ml_dtypes.bfloat16

H = 32
PD = 16
SEQ = 25
BATCH = 262144
NCORES = 8

# per-core decomposition
B_LOC = BATCH // NCORES          # 32768
RUNS = 8
NPACK = 2                        # packs per run
NB = 512                         # batch cols per strip per pack
B_RUN = 4 * NPACK * NB           # 8192 rows per run


def _f32(x):
    return np.ascontiguousarray(np.asarray(x, dtype=np.float32))


def prep_weights(W_ih, W_hh, b_ih, b_hh, W_out, b_out):
    """Host-side weight preprocessing -> small dram params (replicated per strip)."""
    W_ih, W_hh, b_ih, b_hh, W_out, b_out = map(
        _f32, (W_ih, W_hh, b_ih, b_hh, W_out, b_out)
    )
    b1 = b_ih + b_hh                       # step-1 bias [4H]
    W_eff = W_ih @ W_out + W_hh            # [4H, H]
    b_eff = b1 + W_ih @ b_out              # [4H]

    def rep4(mat):  # [K, 128] -> [128, 128] strip-replicated, bf16
        return np.ascontiguousarray(np.tile(mat, (4, 1)).astype(bf16))

    weff = rep4(W_eff.T)                               # [32,128] -> [128,128]
    whh = rep4(W_hh.T)
    wih_pad = np.zeros((H, 4 * H), np.float32)
    wih_pad[:PD] = W_ih.T                              # [16,128] padded to [32,128]
    wih = rep4(wih_pad)
    wout = np.ascontiguousarray(np.tile(W_out.T, (4, 1)).astype(bf16))  # [128,16]

    # bias dram param [128, 8] f32: cols 0-3 = b1 per gate type, 4-7 = b_eff
    bias = np.zeros((128, 8), np.float32)
    for ty in range(4):
        bias[:, ty] = np.tile(b1[32 * ty : 32 * ty + 32], 4)
        bias[:, 4 + ty] = np.tile(b_eff[32 * ty : 32 * ty + 32], 4)
    return dict(weff=weff, whh=whh, wih=wih, wout=wout, bias=bias,
                b_out=b_out)


def prep_state(arr, runs, npack, feat):
    """[B_loc, feat<=32] batch-major -> strip layout [128, runs*npack*NB] bf16.

    partition 32x+k = feature k of strip x; col r*(npack*NB) + p*NB + j
    = batch row r*B_RUN + x*(npack*NB) + p*NB + j.
    """
    b_loc = arr.shape[0]
    a = np.zeros((b_loc, H), np.float32)
    a[:, : arr.shape[1]] = arr
    a = a.reshape(runs, 4, npack, NB, H)          # [r, x, p, j, k]
    a = a.transpose(1, 4, 0, 2, 3)                # [x, k, r, p, j]
    return np.ascontiguousarray(a.reshape(128, runs * npack * NB).astype(bf16))


def build_nc(runs=RUNS, npack=NPACK, seq=SEQ, unroll_steps=False):
    import concourse.bass as bass
    import concourse.bacc as bacc
    import concourse.mybir as mybir
    import concourse.tile as tile

    F32 = mybir.dt.float32
    BF16 = mybir.dt.bfloat16
    AF = mybir.ActivationFunctionType
    b_loc = runs * 4 * npack * NB
    C = npack * NB                     # h/c cols per run
    MB = (4 * npack * NB) // 128       # m-blocks per run (batch rows / 128)

    nc = bacc.Bacc("TRN2", target_bir_lowering=False, debug=False)
    hT_d = nc.declare_dram_parameter("hT", [128, runs * C], BF16, isOutput=False)
    cT_d = nc.declare_dram_parameter("cT", [128, runs * C], BF16, isOutput=False)
    xT_d = nc.declare_dram_parameter("xT", [128, runs * C], BF16, isOutput=False)
    weff_d = nc.declare_dram_parameter("weff", [128, 128], BF16, isOutput=False)
    whh_d = nc.declare_dram_parameter("whh", [128, 128], BF16, isOutput=False)
    wih_d = nc.declare_dram_parameter("wih", [128, 128], BF16, isOutput=False)
    wout_d = nc.declare_dram_parameter("wout", [128, PD], BF16, isOutput=False)
    bias_d = nc.declare_dram_parameter("bias", [128, 8], F32, isOutput=False)
    out_d = nc.declare_dram_parameter("out", [b_loc, seq * PD], F32, isOutput=True)

    GATE_FUNC = [AF.Sigmoid, AF.Sigmoid, AF.Tanh, AF.Sigmoid]  # i, f, g, o

    with tile.TileContext(nc) as tc:
        with (
            tc.tile_pool(name="const", bufs=1) as const,
            tc.tile_pool(name="state", bufs=2) as state,
            tc.tile_pool(name="sig", bufs=2) as sig,
            tc.tile_pool(name="gpsum", bufs=4, space=bass.MemorySpace.PSUM) as gpsum,
            tc.tile_pool(name="xpsum", bufs=1, space=bass.MemorySpace.PSUM) as xpsum,
        ):
            weff_t = const.tile([128, 128], BF16)
            whh_t = const.tile([128, 128], BF16)
            wih_t = const.tile([128, 128], BF16)
            wout_t = const.tile([128, PD], BF16)
            bias_t = const.tile([128, 8], F32)
            nc.sync.dma_start(weff_t[:], weff_d[:])
            nc.sync.dma_start(whh_t[:], whh_d[:])
            nc.sync.dma_start(wih_t[:], wih_d[:])
            nc.sync.dma_start(wout_t[:], wout_d[:])
            nc.sync.dma_start(bias_t[:], bias_d[:])

            for r in range(runs):
                h_sb = state.tile([128, C], BF16)
                c_sb = state.tile([128, C], BF16)
                x0_sb = state.tile([128, C], BF16)
                xs = state.tile([128, MB * seq * PD], F32)
                nc.sync.dma_start(h_sb[:], hT_d[:, r * C : (r + 1) * C])
                nc.sync.dma_start(c_sb[:], cT_d[:, r * C : (r + 1) * C])
                nc.sync.dma_start(x0_sb[:], xT_d[:, r * C : (r + 1) * C])

                # x_stage view [p][x][pp=pack][q][seq*PD]
                xs_v = xs[:].rearrange(
                    "p (xx pp qq c) -> p xx pp qq c",
                    xx=4, pp=npack, qq=4, c=seq * PD,
                )

                def do_step(step0, xcol, biascol):
                    """one LSTM step for all packs. xcol = dyn/static col offset
                    (units of f32 elements) into the per-m-block 400-col region."""
                    X = xpsum.tile([128, 2048], F32, name="X")
                    for p in range(npack):
                        cs = slice(p * NB, (p + 1) * NB)
                        # ---- phase A: gate matmuls (diagonal tiles) ----
                        gb = [gpsum.tile([128, NB], F32, name="gb")
                              for ty in range(4)]
                        for ty in range(4):
                            for x in range(4):
                                ps = slice(32 * x, 32 * x + 32)
                                ws = slice(32 * ty, 32 * ty + 32)
                                if step0:
                                    nc.tensor.matmul(
                                        gb[ty][ps, :], whh_t[ps, ws], h_sb[ps, cs],
                                        start=True, stop=False,
                                        tile_position=(32 * x, 32 * x),
                                    )
                                    nc.tensor.matmul(
                                        gb[ty][ps, :], wih_t[ps, ws], x0_sb[ps, cs],
                                        start=False, stop=True,
                                        tile_position=(32 * x, 32 * x),
                                    )
                                else:
                                    nc.tensor.matmul(
                                        gb[ty][ps, :], weff_t[ps, ws], h_sb[ps, cs],
                                        start=True, stop=True,
                                        tile_position=(32 * x, 32 * x),
                                    )
                        # ---- gate activations (bias folded in) ----
                        S = []
                        for ty in range(4):
                            s_t = sig.tile([128, NB], BF16, name=f"s{ty}")
                            nc.scalar.activation(
                                s_t[:], gb[ty][:],
                                GATE_FUNC[ty],
                                bias=bias_t[:, biascol + ty : biascol + ty + 1],
                            )
                            S.append(s_t)
                        s_i, s_f, s_g, s_o = S
                        # ---- cell update (DVE, bf16 2x) ----
                        t1 = sig.tile([128, NB], BF16, name="t1")
                        t2 = sig.tile([128, NB], BF16, name="t2")
                        nc.vector.tensor_mul(t1[:], s_f[:], c_sb[:, cs])
                        nc.vector.tensor_mul(t2[:], s_i[:], s_g[:])
                        nc.vector.tensor_add(c_sb[:, cs], t1[:], t2[:])
                        s_tc = sig.tile([128, NB], BF16, name="stc")
                        nc.scalar.activation(s_tc[:], c_sb[:, cs], AF.Tanh)
                        nc.vector.tensor_mul(h_sb[:, cs], s_o[:], s_tc[:])
                        # ---- phase B: projection, batch-major out ----
                        for x in range(4):
                            ps = slice(32 * x, 32 * x + 32)
                            base = 512 * x + 64 * p
                            for q in range(4):
                                nc.tensor.matmul(
                                    X[:, base + PD * q : base + PD * (q + 1)],
                                    h_sb[ps, p * NB + 128 * q : p * NB + 128 * (q + 1)],
                                    wout_t[ps, :],
                                    start=True, stop=True,
                                    tile_position=(32 * x, 0),
                                )
                    # one staging copy per step: [x][p][q][16] -> x_stage blocks
                    xsrc = X[:].rearrange("p (x r) -> p x r", x=4)[
                        :, :, 0 : 64 * npack
                    ].rearrange("p x (pp qq c) -> p x pp qq c", pp=npack, qq=4, c=PD)
                    xdst = xs_v[:, :, :, :, bass.ds(xcol, PD)]
                    nc.vector.tensor_copy(xdst, xsrc)

                do_step(True, 0, 0)
                if unroll_steps:
                    for t in range(1, seq):
                        do_step(False, t * PD, 4)
                else:
                    with tc.For_i(PD, seq * PD, PD) as iv:
                        do_step(False, iv, 4)

                # ---- flush run output ----
                od = out_d[r * (128 * MB) : (r + 1) * (128 * MB), :].rearrange(
                    "(m jj) c -> jj m c", jj=128
                )
                nc.sync.dma_start(od, xs[:].rearrange("p (m c) -> p m c", c=seq * PD))
    nc.compile()
    return nc


_NC_CACHE = {}


def _get_nc(key=("full",)):
    if key not in _NC_CACHE:
        _NC_CACHE[key] = build_nc(unroll_steps=True)
    return _NC_CACHE[key]


def make_in_maps(inputs):
    """host-side prep: full inputs dict -> (in_maps list per core, b_out)."""
    first_input = _f32(inputs["first_input"])
    h0 = _f32(inputs["h0"])
    c0 = _f32(inputs["c0"])
    w = prep_weights(
        inputs["W_ih"], inputs["W_hh"], inputs["b_ih"], inputs["b_hh"],
        inputs["W_out"], inputs["b_out"],
    )
    shared = dict(
        weff=w["weff"], whh=w["whh"], wih=w["wih"], wout=w["wout"], bias=w["bias"]
    )
    in_maps = []
    for ci in range(NCORES):
        rows = slice(ci * B_LOC, (ci + 1) * B_LOC)
        in_maps.append(dict(
            shared,
            hT=prep_state(h0[rows], RUNS, NPACK, H),
            cT=prep_state(c0[rows], RUNS, NPACK, H),
            xT=prep_state(first_input[rows], RUNS, NPACK, PD),
        ))
    return in_maps, w["b_out"]


def kernel(**inputs) -> np.ndarray:
    from concourse.bass_utils import run_bass_kernel_spmd

    in_maps, b_out = make_in_maps(inputs)
    nc = _get_nc()
    res = run_bass_kernel_spmd(nc, in_maps, core_ids=list(range(NCORES)))
    outs = [res.results[i]["out"].reshape(B_LOC, SEQ, PD) for i in range(NCORES)]
    full = np.concatenate(outs, axis=0).astype(np.float32)
    full += b_out[None, None, :]
    return full


if __name__ == "__main__":
    # smoke: build the full nc and report instruction counts
    nc = build_nc()
    n = sum(len(b.instructions) for b in nc.m.functions[0].blocks)
    print("built; instructions:", n)

